# revision 37
# baseline (speedup 1.0000x reference)
"""GQA attention block (QKV proj + RoPE + causal attention + out proj),
tensor-parallel over 8 TRN2 NeuronCores.

Sharding / schedule (all fp16 on the PE, fp32 PSUM accumulation):
- Phase 1 (QKV+RoPE): heads split across cores (4 q-heads + 1 kv-head per
  core); wq/wk/wv column-split by head; every core reads the full
  (pre-transposed, fp16) x.  Emitted per 512-token chunk.
- Phase 2 (attention): per-chunk, pipelined right behind phase 1 (chunk qc
  only needs K/V chunks <= qc).  Causal structure exploited two ways:
  fully-masked 128-key tiles are skipped, and diagonal tiles only compute
  the valid q-suffix (triangular score matmuls + narrowed exp).  Pairs of
  unmasked score tiles share one 2-bank PSUM tile and a single [128,1024]
  exp to amortize ACT overhead.  The softmax denominator is accumulated on
  DVE (fp16) and reduced across partitions on GpSimd
  (partition_all_reduce), keeping the PE free for score/PV matmuls.  The
  next chunk's projection matmuls are emitted as micro-ops interleaved into
  the attention stream so the PE stays fed while ACT runs exps.
- Phase 3 (out proj): token-sharded.  Per-head attention outputs are
  redistributed with one small AllToAll per batch (feature-sharded ->
  token-sharded), each launched as soon as its batch's attention finishes so
  both hide under compute (this replaces two fully-exposed AllGathers,
  ~530us, in the old feature-sharded design).  Each core then multiplies
  its 2x256-token slice by the FULL wo, streamed from HBM through a 2-slot
  SBUF ring with 4-way-split loads.  The phase-1 pools close before the
  last attention chunk so phase 3's pools can open early: batch-0's whole
  out-projection is emitted as micro-ops that fill the last (heaviest)
  attention chunk's ACT-bound stalls, and batch 1 walks the out-feature
  chunks in reverse to reuse the ring tiles without a reload.

Other details: RoPE pair layout is pre-permuted on the host (even/odd ->
halves) so rotation is 6 DVE ops per tile; the q-side rope table also folds
in the 1/sqrt(HD) score scale; V is transposed on-chip via DMA transpose;
mask patterns are classified host-side into dedup'd additive tiles.
"""

import sys
from contextlib import ExitStack

sys.path.insert(0, "/opt/trn_rl_repo")

import numpy as np

import concourse.bass as bass
import concourse.bass_isa as bass_isa
import concourse.mybir as mybir
import concourse.tile as tile
from concourse import bacc
from concourse.bass_utils import run_bass_kernel_spmd

N_CORES = 8
B = 2
DIM = 4096
H = 32
KVH = 8
HD = 128
HL = H // N_CORES          # 4 local q-heads
KC = DIM // 128            # 32 contraction chunks
FP16 = mybir.dt.float16
FP32 = mybir.dt.float32

SKIP, PLAIN = -1, 0


def classify_mask(mask):
    """Per (512-q-chunk, 128-k-tile) block classification of the additive mask.

    Returns (cls, pats):
      cls[j][kt] = (kind, qlo, pw) where kind in {SKIP, PLAIN, 1+pat_idx};
        qlo: first q column (0..511) of the chunk with any unmasked k in this
             block (columns < qlo are fully masked and are skipped by the
             triangular score matmul);
        pw:  width of the column range [qlo, qlo+pw) that needs the additive
             pattern applied (0 for PLAIN).
      pats: list of distinct [128, 512] fp32 additive mask tiles (mask.T
        blocks), added to the scores before exp.
    """
    S = mask.shape[0]
    nch, nkt = S // 512, S // 128
    m = mask.astype(np.float32)
    em = np.exp(m.astype(np.float64))
    cls = [[None] * nkt for _ in range(nch)]
    pats = []
    keys = {}
    for j in range(nch):
        for kt in range(nkt):
            qs, ks = slice(j * 512, (j + 1) * 512), slice(kt * 128, (kt + 1) * 128)
            blk = m[qs, ks]          # [512q, 128k]
            eblk = em[qs, ks]
            if not eblk.any():
                cls[j][kt] = (SKIP, 0, 0)
            elif (blk == 0.0).all():
                cls[j][kt] = (PLAIN, 0, 0)
            else:
                valid = eblk.any(axis=1)       # per-q: any unmasked k
                qlo = int(np.argmax(valid))
                if not valid[qlo:].all():
                    qlo = 0                    # non-suffix valid region: full
                nzcols = (blk != 0.0).any(axis=1)
                nzcols[:qlo] = False
                if nzcols.any():
                    pw = int(np.nonzero(nzcols)[0].max()) + 1 - qlo
                else:
                    pw = 0
                pat = np.ascontiguousarray(blk.T)  # [128k, 512q] fp32
                k = pat.tobytes()
                if k not in keys:
                    keys[k] = len(pats)
                    pats.append(pat)
                cls[j][kt] = (1 + keys[k], qlo, pw)
    return cls, pats


def build(S, cls, npat, n_iters=1):
    nch = S // 512   # 512-token chunks per batch
    nkt = S // 128   # 128-token k tiles per batch
    TPC = S // N_CORES          # tokens per core (per batch) in phase 3
    NOF = DIM // 512            # 512-wide out-feature chunks
    ntt = TPC // 128            # 128-token tiles per core per batch

    nc = bacc.Bacc("TRN2", target_bir_lowering=False, debug=False,
                   num_devices=N_CORES)

    xT = nc.declare_dram_parameter("xT", [B, nch, 128, KC, 512], FP16, isOutput=False)
    wqT = nc.declare_dram_parameter("wqT", [128, KC, HL * HD], FP16, isOutput=False)
    wkT = nc.declare_dram_parameter("wkT", [128, KC, HD], FP16, isOutput=False)
    wvT = nc.declare_dram_parameter("wvT", [128, KC, HD], FP16, isOutput=False)
    woT = nc.declare_dram_parameter("woT", [NOF, 128, KC, 512], FP16, isOutput=False)
    csP = nc.declare_dram_parameter("cs", [128, S], FP16, isOutput=False)
    csqP = nc.declare_dram_parameter("csq", [128, S], FP16, isOutput=False)
    patP = nc.declare_dram_parameter("pats", [128, max(npat, 1) * 512], FP32,
                                     isOutput=False)
    outP = nc.declare_dram_parameter("out", [B, TPC, DIM], FP16, isOutput=True)

    with tile.TileContext(nc) as tc:
        with tc.tile_pool(name="dram", bufs=1, space="DRAM") as dram:
            for _it in range(n_iters):
                a2a_in, a2a_out = [], []
                for b in range(B):
                    a2a_in.append(dram.tile([N_CORES, HL, 128, TPC], FP16,
                                            name=f"a2ain{b}_{_it}",
                                            tag=f"a2ain{b}_{_it}"))
                    a2a_out.append(dram.tile([N_CORES, HL, 128, TPC], FP16,
                                             name=f"a2aout{b}_{_it}",
                                             tag=f"a2aout{b}_{_it}"))

                with ExitStack() as ph2_ctx:
                    qkv = ph2_ctx.enter_context(tc.tile_pool(name="qkv", bufs=1))
                    patpool = ph2_ctx.enter_context(tc.tile_pool(name="patpool", bufs=1))
                    stps = ph2_ctx.enter_context(tc.tile_pool(name="stps", bufs=2, space="PSUM"))
                    atps = ph2_ctx.enter_context(tc.tile_pool(name="atps", bufs=2, space="PSUM"))
                    ptp = ph2_ctx.enter_context(tc.tile_pool(name="ptp", bufs=4))
                    rsp = ph2_ctx.enter_context(tc.tile_pool(name="rsp", bufs=2))
                    attnp = ph2_ctx.enter_context(tc.tile_pool(name="attnp", bufs=4))
                    smallp = ph2_ctx.enter_context(tc.tile_pool(name="smallp", bufs=2))
                    ph2_ctx2 = ph2_ctx

                    # phase-1 outputs (resident through phase 2)
                    qT_sb = qkv.tile([128, B, HL, S], FP16)       # [d, b, h, t]
                    kT_sb = qkv.tile([128, B, S], FP16)           # [d, b, t]
                    v_sb = qkv.tile([128, B, nkt, HD], FP16)      # [t%128, b, kt, d]

                    pats_sb = patpool.tile([128, max(npat, 1) * 512], FP32)
                    nc.sync.dma_start(out=pats_sb[:], in_=patP[:, :])

                    def ph2_qc(b, qc, filler=()):
                        """Attention for one 512-token q chunk (all local heads).
                        Needs phase-1 chunks 0..qc of batch b only; emission of
                        the next chunk's projection jobs is interleaved after
                        each head so the PE stays fed while ACT runs exps."""
                        q0 = qc * 512
                        fill = {"i": 0}

                        def pull(n):
                            i = fill["i"]
                            while n > 0 and i < len(filler):
                                filler[i]()
                                i += 1
                                n -= 1
                            fill["i"] = i
                        ktlist = [(kt,) + cls[qc][kt][1:] + (cls[qc][kt][0],)
                                  for kt in range(nkt) if cls[qc][kt][0] != SKIP]
                        assert ktlist[0][1] == 0, "first k-tile must be full-width"
                        last_i = len(ktlist) - 1
                        # group consecutive PLAIN k-tiles into pairs: their
                        # scores land in the two halves of one 2-bank PSUM
                        # tile and share a single [128,1024] exp.
                        groups = []
                        i = 0
                        while i < len(ktlist):
                            a = ktlist[i]
                            if (i + 1 < len(ktlist) and a[3] == PLAIN
                                    and ktlist[i + 1][3] == PLAIN):
                                groups.append((a[0], ktlist[i + 1][0], None))
                                i += 2
                            else:
                                groups.append((a[0], None, a))
                                i += 1
                        for h in range(HL):
                            at = atps.tile([128, 512], FP32, tag="at")
                            rsacc = rsp.tile([128, 512], FP16, tag="rsacc")
                            # 1-deep software pipeline: group g+1's scores/exp
                            # are emitted before group g's PV so the PE never
                            # sits on an exp-wait (stps bufs=2 holds both).
                            istate = {"i": 0}
                            deferred = None

                            def stage1(kt0, kt1, single):
                                st = stps.tile([128, 1024], FP32, tag="st")
                                pt = ptp.tile([128, 1024], FP16, tag="pt")
                                if single is None:
                                    for half, kt in ((0, kt0), (1, kt1)):
                                        nc.tensor.matmul(
                                            st[:, half * 512:(half + 1) * 512],
                                            lhsT=kT_sb[:, b, kt * 128:(kt + 1) * 128],
                                            rhs=qT_sb[:, b, h, q0:q0 + 512],
                                            start=True, stop=True)
                                    nc.scalar.activation(
                                        pt[:], st[:],
                                        mybir.ActivationFunctionType.Exp)

                                    def pv():
                                        pull(2)
                                        for half, kt in ((0, kt0), (1, kt1)):
                                            ph = pt[:, half * 512:(half + 1) * 512]
                                            i = istate["i"]
                                            nc.tensor.matmul(
                                                at[:], lhsT=v_sb[:, b, kt, :],
                                                rhs=ph,
                                                start=(i == 0), stop=(i == last_i))
                                            if i == 0:
                                                nc.vector.tensor_copy(rsacc[:], ph)
                                            else:
                                                nc.vector.tensor_add(
                                                    rsacc[:], rsacc[:], ph)
                                            istate["i"] = i + 1
                                    return pv
                                kt, qlo, pw, kind = single
                                nc.tensor.matmul(
                                    st[:, qlo:512],
                                    lhsT=kT_sb[:, b, kt * 128:(kt + 1) * 128],
                                    rhs=qT_sb[:, b, h, q0 + qlo:q0 + 512],
                                    start=True, stop=True)
                                if kind > 0 and pw > 0:
                                    p0 = (kind - 1) * 512 + qlo
                                    nc.vector.tensor_add(
                                        st[:, qlo:qlo + pw], st[:, qlo:qlo + pw],
                                        pats_sb[:, p0:p0 + pw])
                                if qlo > 0:
                                    nc.vector.memset(pt[:, 0:qlo], 0.0)
                                nc.scalar.activation(
                                    pt[:, qlo:512], st[:, qlo:512],
                                    mybir.ActivationFunctionType.Exp)

                                def pv():
                                    pull(2)
                                    i = istate["i"]
                                    nc.tensor.matmul(
                                        at[:], lhsT=v_sb[:, b, kt, :],
                                        rhs=pt[:, 0:512],
                                        start=(i == 0), stop=(i == last_i))
                                    if i == 0:
                                        nc.vector.tensor_copy(rsacc[:], pt[:, 0:512])
                                    else:
                                        nc.vector.tensor_add(
                                            rsacc[:, qlo:512], rsacc[:, qlo:512],
                                            pt[:, qlo:512])
                                    istate["i"] = i + 1
                                return pv

                            for kt0, kt1, single in groups:
                                cur = stage1(kt0, kt1, single)
                                if deferred is not None:
                                    deferred()
                                deferred = cur
                            deferred()
                            rs = smallp.tile([128, 512], FP32, tag="rs")
                            nc.gpsimd.partition_all_reduce(
                                rs[:], rsacc[:], 128, bass_isa.ReduceOp.add)
                            bcs = smallp.tile([128, 512], FP32, tag="bcs")
                            nc.vector.reciprocal(bcs[:], rs[:])
                            attn = attnp.tile([128, 512], FP16, tag="attn")
                            nc.vector.tensor_mul(attn[:], at[:], bcs[:])
                            nds = 512 // TPC   # destination cores per q chunk
                            for piece in range(nds):
                                nc.sync.dma_start(
                                    out=a2a_in[b][qc * nds + piece, h],
                                    in_=attn[:, piece * TPC:(piece + 1) * TPC])
                        pull(len(filler))

                    def a2a(b):
                        nc.gpsimd.collective_compute(
                            "AllToAll", mybir.AluOpType.bypass,
                            replica_groups=[list(range(N_CORES))],
                            ins=[a2a_in[b].opt()], outs=[a2a_out[b].opt()])

                    # ---------------- phase 1+2 pipelined per chunk ----------------
                    with ExitStack() as ph1_ctx:
                        wpool = ph1_ctx.enter_context(tc.tile_pool(name="wpool", bufs=1))
                        cspool = ph1_ctx.enter_context(tc.tile_pool(name="cspool", bufs=1))
                        xtp = ph1_ctx.enter_context(tc.tile_pool(name="xt", bufs=4))
                        ph1ps = ph1_ctx.enter_context(tc.tile_pool(name="ph1ps", bufs=2, space="PSUM"))
                        ph1tmp = ph1_ctx.enter_context(tc.tile_pool(name="ph1tmp", bufs=1))
                        del ph1_ctx

                        def load_xt(b, tcn, halves=(0, 1)):
                            xts = []
                            for half in halves:
                                xt_h = xtp.tile([128, KC // 2, 512], FP16, tag="xt")
                                nc.sync.dma_start(
                                    out=xt_h[:],
                                    in_=xT[b, tcn, :, half * (KC // 2):(half + 1) * (KC // 2), :])
                                xts.append(xt_h)
                            return xts

                        # First compute needs: wk + xt(b0,half0) -> emit
                        # those DMAs first (xt half split in 4 sub-loads) so
                        # the PE can start ASAP; wq per-head.
                        wq_sb = wpool.tile([128, KC, HL * HD], FP16)
                        wk_sb = wpool.tile([128, KC, HD], FP16)
                        wv_sb = wpool.tile([128, KC, HD], FP16)
                        xt0_h0 = xtp.tile([128, KC // 2, 512], FP16, tag="xt")
                        # interleave wk / first-xt quarter loads so the first
                        # k-projection matmul can issue after ~2 quarter DMAs
                        for cq in range(4):
                            wr = slice(cq * (KC // 4), (cq + 1) * (KC // 4))
                            nc.sync.dma_start(out=wk_sb[:, wr], in_=wkT[:, wr])
                            cr = slice(cq * (KC // 8), (cq + 1) * (KC // 8))
                            nc.sync.dma_start(out=xt0_h0[:, cr],
                                              in_=xT[0, 0, :, cr, :])
                        xts_next = [xt0_h0]
                        xts_next = xts_next + load_xt(0, 0, (1,))
                        nc.sync.dma_start(out=wv_sb[:], in_=wvT[:, :, :])
                        cs_sb = cspool.tile([128, S], FP16)
                        csq_sb = cspool.tile([128, S], FP16)
                        nc.sync.dma_start(out=cs_sb[:], in_=csP[:, :])
                        for h in range(HL):
                            nc.sync.dma_start(out=wq_sb[:, :, h * HD:(h + 1) * HD],
                                              in_=wqT[:, :, h * HD:(h + 1) * HD])
                        nc.sync.dma_start(out=csq_sb[:], in_=csqP[:, :])

                        def rope_evict(ps, dst, cst, t0):
                            c = cst[0:64, t0:t0 + 512]
                            s = cst[64:128, t0:t0 + 512]
                            qE = ps[0:64, :]
                            qO = ps[64:128, :]
                            t1 = ph1tmp.tile([64, 512], FP32, tag="t1")
                            t2 = ph1tmp.tile([64, 512], FP32, tag="t2")
                            t3 = ph1tmp.tile([64, 512], FP32, tag="t3")
                            t4 = ph1tmp.tile([64, 512], FP32, tag="t4")
                            nc.vector.tensor_mul(t1[:], qE, c)
                            nc.vector.tensor_mul(t2[:], qO, s)
                            nc.vector.tensor_sub(dst[0:64, :], t1[:], t2[:])
                            nc.vector.tensor_mul(t3[:], qE, s)
                            nc.vector.tensor_mul(t4[:], qO, c)
                            nc.vector.tensor_add(dst[64:128, :], t3[:], t4[:])

                        def ph1_jobs(b, tcn, xts):
                            """The 6 projection jobs (k, v, q0..q3) for one
                            chunk, as thunks to interleave into the previous
                            chunk's attention (fills ACT-bound PE gaps)."""
                            t0 = tcn * 512

                            def proj(w_sb, n0, psum):
                                for c in range(KC):
                                    nc.tensor.matmul(
                                        psum[:],
                                        lhsT=w_sb[:, c, n0:n0 + 128],
                                        rhs=xts[c // (KC // 2)][:, c % (KC // 2), :],
                                        start=(c == 0), stop=(c == KC - 1))

                            state = {}

                            def mm(w_sb, n0, c, key):
                                def f():
                                    if c == 0:
                                        state[key] = ph1ps.tile(
                                            [128, 512], FP32, tag="qk", name="pp")
                                    nc.tensor.matmul(
                                        state[key][:],
                                        lhsT=w_sb[:, c, n0:n0 + 128],
                                        rhs=xts[c // (KC // 2)][:, c % (KC // 2), :],
                                        start=(c == 0), stop=(c == KC - 1))
                                return f

                            def kevict():
                                rope_evict(state["k"], kT_sb[:, b, t0:t0 + 512],
                                           cs_sb, t0)

                            def vevict():
                                vt_tmp = ph1tmp.tile([128, 512], FP16, tag="vt")
                                nc.scalar.copy(vt_tmp[:], state["v"][:])
                                for tt in range(4):
                                    nc.sync.dma_start_transpose(
                                        out=v_sb[:, b, tcn * 4 + tt, :],
                                        in_=vt_tmp[:, tt * 128:(tt + 1) * 128])

                            def qevict(h):
                                def f():
                                    rope_evict(state["q%d" % h],
                                               qT_sb[:, b, h, t0:t0 + 512],
                                               csq_sb, t0)
                                return f

                            ops = []
                            ops += [mm(wk_sb, 0, c, "k") for c in range(KC)]
                            ops.append(kevict)
                            ops += [mm(wv_sb, 0, c, "v") for c in range(KC)]
                            ops.append(vevict)
                            for h in range(HL):
                                ops += [mm(wq_sb, h * HD, c, "q%d" % h)
                                        for c in range(KC)]
                                ops.append(qevict(h))
                            return ops

                        chunks = [(b, tcn) for b in range(B) for tcn in range(nch)]
                        for ci, (b, tcn) in enumerate(chunks[:-1]):
                            if ci == 0:
                                for job in ph1_jobs(b, tcn, xts_next):
                                    job()
                            nb, ntc = chunks[ci + 1]
                            nxt_jobs = ph1_jobs(nb, ntc, load_xt(nb, ntc))
                            ph2_qc(b, tcn, nxt_jobs)
                            if tcn == nch - 1:
                                a2a(b)
                        # chunk (B-1, nch-1)'s projections were interleaved
                        # above; its attention runs after the phase-1 pools
                        # close so phase-3 preloads + pass-0 matmuls can fill
                        # its ACT-bound stalls.

                    # ------------- phase 3: out projection -------------
                    # (inside the ph2 scope: pass 0 interleaves with the last
                    # attention chunk; ph1 pools are closed by now.)
                    wop = ph2_ctx2.enter_context(tc.tile_pool(name="wop", bufs=1))
                    a2ap = ph2_ctx2.enter_context(tc.tile_pool(name="a2ap", bufs=1))
                    outp = ph2_ctx2.enter_context(tc.tile_pool(name="outp", bufs=3))
                    ph3ps = ph2_ctx2.enter_context(tc.tile_pool(name="ph3ps", bufs=2, space="PSUM"))
                    NWO = 2
                    wo_ring = [wop.tile([128, KC, 512], FP16, name=f"wo{i}",
                                        tag=f"wo{i}") for i in range(NWO)]
                    wo_loaded = {}

                    def wo_load(ofc):
                        if wo_loaded.get(ofc % NWO) == ofc:
                            return
                        for cq in range(4):
                            cr = slice(cq * (KC // 4), (cq + 1) * (KC // 4))
                            nc.sync.dma_start(out=wo_ring[ofc % NWO][:, cr],
                                              in_=woT[ofc][:, cr])
                        wo_loaded[ofc % NWO] = ofc

                    def a2a_load(b, tags):
                        sb_tt = []
                        for tt in range(ntt):
                            t = a2ap.tile([128, N_CORES, HL, 128], FP16,
                                          name=f"a2asb{tags[tt]}",
                                          tag=f"a2asb{tags[tt]}")
                            sb_tt.append(t)
                            for sh in range(2):
                                sr = slice(sh * (N_CORES // 2), (sh + 1) * (N_CORES // 2))
                                nc.sync.dma_start(
                                    out=t[:, sr],
                                    in_=a2a_out[b][sr, :, :, tt * 128:(tt + 1) * 128]
                                        .rearrange("s h d t -> d s h t"))
                        return sb_tt

                    def ph3_ops(b, sb_tt, ofcs):
                        """Pass-b out-projection as micro-ops (one matmul per
                        thunk) so it can fill attention stalls."""
                        ops = []
                        state = {}

                        def ldw(ofc):
                            def f():
                                wo_load(ofc)
                            return f

                        def mm3(ofc, tt, c):
                            def f():
                                if c == 0:
                                    state["ps"] = ph3ps.tile(
                                        [128, 512], FP32, tag="ph3", name="ps3")
                                nc.tensor.matmul(
                                    state["ps"][:],
                                    lhsT=sb_tt[tt][:, c // HL, c % HL, :],
                                    rhs=wo_ring[ofc % NWO][:, c, :],
                                    start=(c == 0), stop=(c == KC - 1))
                            return f

                        def evict(ofc, tt):
                            def f():
                                ot = outp.tile([128, 512], FP16, tag="o")
                                nc.scalar.copy(ot[:], state["ps"][:])
                                nc.sync.dma_start(
                                    out=outP[b, tt * 128:(tt + 1) * 128,
                                             ofc * 512:(ofc + 1) * 512],
                                    in_=ot[:])
                            return f

                        for idx, ofc in enumerate(ofcs):
                            ops.append(ldw(ofc))
                            for tt in range(ntt):
                                ops += [mm3(ofc, tt, c) for c in range(KC)]
                                ops.append(evict(ofc, tt))
                            if idx + NWO < len(ofcs):
                                ops.append(ldw(ofcs[idx + NWO]))
                        return ops

                    sb0 = a2a_load(0, ("A", "B"))
                    wo_load(0)
                    wo_load(1)
                    lb, ltc = chunks[-1]
                    ph2_qc(lb, ltc, ph3_ops(0, sb0, list(range(NOF))))
                    a2a(lb)
                    sb1 = a2a_load(1, ("C", "B"))
                    for job in ph3_ops(1, sb1, list(range(NOF - 1, -1, -1))):
                        job()
    nc.compile()
    return nc


def make_inputs(x, wq, wk, wv, wo, freqs_cos, freqs_sin, mask):
    """Host-side sharding/transposes. Returns (in_maps, cls, npat, S)."""
    S = x.shape[1]
    nch = S // 512
    perm = np.concatenate([np.arange(0, HD, 2), np.arange(1, HD, 2)])
    x_ = np.asarray(x, dtype=np.float32)
    # [B, nch, 128, KC, 512]: element (b,tcn,p,c,t) = x[b, tcn*512+t, c*128+p]
    xTn = np.ascontiguousarray(
        x_.reshape(B, nch, 512, KC, 128).transpose(0, 1, 4, 3, 2)).astype(np.float16)
    cs64 = np.ascontiguousarray(
        np.concatenate([np.asarray(freqs_cos).T, np.asarray(freqs_sin).T], axis=0)
    ).astype(np.float64)
    cs = cs64.astype(np.float16)
    csq = (cs64 * (1.0 / np.sqrt(HD))).astype(np.float16)
    cls, pats = classify_mask(np.asarray(mask, dtype=np.float32))
    npat = len(pats)
    if npat:
        patA = np.ascontiguousarray(np.concatenate(list(pats), axis=1)).astype(np.float32)
    else:
        patA = np.zeros((128, 512), np.float32)
    wq_, wk_, wv_, wo_ = (np.asarray(a, dtype=np.float32) for a in (wq, wk, wv, wo))

    def chunkT(w_rows):
        # w_rows: [n, DIM] -> lhsT chunks [128, KC, n]
        n = w_rows.shape[0]
        return np.ascontiguousarray(
            w_rows.T.reshape(KC, 128, n).transpose(1, 0, 2)).astype(np.float16)

    # full wo, tiled by 512-wide out-feature chunks: [NOF, 128, KC, 512]
    # element (o, p, c, n) = wo[o*512+n, c*128+p]
    woTn = np.ascontiguousarray(
        wo_.T.reshape(KC, 128, DIM // 512, 512).transpose(2, 1, 0, 3)).astype(np.float16)

    in_maps = []
    for c in range(N_CORES):
        qrows = np.concatenate([c * (HL * HD) + h * HD + perm for h in range(HL)])
        krows = c * HD + perm
        in_maps.append({
            "xT": xTn,
            "wqT": chunkT(wq_[qrows]),
            "wkT": chunkT(wk_[krows]),
            "wvT": chunkT(wv_[c * HD:(c + 1) * HD]),
            "woT": woTn,
            "cs": cs,
            "csq": csq,
            "pats": patA,
        })
    return in_maps, cls, npat, S


_build_cache = {}


def kernel(x, wq, wk, wv, wo, freqs_cos, freqs_sin, mask, start_pos=0, **_):
    in_maps, cls, npat, S = make_inputs(x, wq, wk, wv, wo, freqs_cos, freqs_sin, mask)
    key = (S, npat, str(cls))
    if key not in _build_cache:
        _build_cache[key] = build(S, cls, npat)
    nc = _build_cache[key]
    res = run_bass_kernel_spmd(nc, in_maps, core_ids=list(range(N_CORES)), trace=False)
    TPC = S // N_CORES
    outs = [res.results[c]["out"].reshape(B, TPC, DIM) for c in range(N_CORES)]
    return np.concatenate(outs, axis=1).astype(np.float32)
